# revision 32
# baseline (speedup 1.0000x reference)
"""BertSelfAttention with gated prompt-prefix branch on 8 Trainium2 cores.

Sharding: data-parallel over batch (B=8 -> 1 batch element per core), no
collectives. Per core the kernel is organized as a software pipeline whose
clock is the ScalarE (ACT) exp spine: 6 head-pairs x 17 exp tiles of
[128, 1024] each (~1.15us per ACT). All other engines are scheduled so the
PE never idles (HAM stays warm) and the ACT never waits:

  window c (one head pair, 16 score beats + prefix):
    PE : score MMs (pair c)            2 MMs/beat, h0+h1 row-concurrent
         ctx MMs (pair c-1)            accumulate [65,1024] psum, K=128
         prefix score/ctx MMs          row-concurrent halves
         projection filler             Q/K/V/prompt chunks woven in gaps
    ACT: exp of score tile (beat b)    the spine, ~100% busy
    DVE: psum evacuations (frees the 2-slot score pool + ctx accumulators),
         projection bias adds, softmax normalization muls
    GpS: final combine adds
    DMA: input staging, denominator reciprocal broadcast, output rows

Scores for a beat pack both heads: st[:, 0:512] = h0, st[:, 512:1024] = h1
(one s-half each) so the two K=64 matmuls occupy disjoint PE row halves and
run concurrently; one ACT covers both. Softmax denominators ride an extra
ones-column in the augmented V (col 65h+64), giving [65,1024] ctx tiles
whose row 64 is sum_t exp. exp(mask) is folded into V rows.

Output is produced as outT [768, 1024] fp32 per core; the host transposes
and stacks to [8, 1024, 768].
"""

import numpy as np
import ml_dtypes

import concourse.bass as bass
import concourse.mybir as mybir
import concourse.tile as tile
from concourse.bass_utils import run_bass_kernel_spmd
from concourse.vector_clock import ScopedClock


class SplitDrainTileContext(tile.TileContext):
    """This walrus build rejects >2 sync waits on the kernel-tail Drain
    ("Too many sync wait commands"); split them across SP nops instead."""

    def _drain_and_barrier(self, tick_clock, wait_clock):
        probe = self.nc.sync.nop(nofuse=True, hint="drain_wait_split")
        wait_clock.add_sem_waits(
            probe.ins, ScopedClock({None: tick_clock.global_clock})
        )
        waits = list(probe.ins.sync_info.on_wait or [])
        if len(waits) > 1:
            probe.ins.sync_info.on_wait = waits[:1]
            for i in range(1, len(waits)):
                extra = self.nc.sync.nop(nofuse=True, hint="drain_wait_split")
                extra.ins.sync_info = mybir.SyncInfo(
                    on_wait=waits[i : i + 1], on_update=[]
                )
        drain_inst = self.nc.sync.drain()
        if drain_inst.ins.sync_info is not None:
            drain_inst.ins.sync_info.on_wait = []
        self.nc.all_engine_barrier()
        assert self.sems is not None
        popped = self.nc._tile_sem_poison_stack.pop()
        assert popped is self._sem_poison
        self.nc.clear_and_free_semaphores(list(self.sems.allocated().values()))
        self.nc.all_engine_barrier()

F32 = mybir.dt.float32
BF16 = mybir.dt.bfloat16
AF = mybir.ActivationFunctionType

H, DH, D = 12, 64, 768
S, AT, B = 1024, 64, 8
SCALE = 1.0 / np.sqrt(DH)
NC_D = D // 128  # 6 chunks over feature dim
NC_S = S // 128  # 8 chunks over sequence dim
PAIRS = H // 2  # 6 head pairs
VW = H * (DH + 1)  # 780: v with per-head ones column

_CACHE = {}
LAST_RESULTS = None


def _split_sync_waits(nc, cap=1):
    """Walrus on this image allows very few sync-wait commands per
    instruction (tensor_scalar rejects 2). Hoist excess waits onto
    same-engine nops placed immediately before the instruction."""
    for bb in nc.main_func.blocks:
        cur = list(bb.instructions)
        out = []
        for inst in cur:
            si = inst.sync_info
            waits = list(si.on_wait) if si and si.on_wait else []
            if len(waits) > cap:
                for i in range(0, len(waits) - cap):
                    bi = nc.engines[inst.engine].nop(
                        nofuse=True, hint="wait_split")
                    popped = nc.cur_bb.bb.instructions.pop()
                    assert popped is bi.ins
                    bi.ins.sync_info = mybir.SyncInfo(
                        on_wait=waits[i : i + 1], on_update=[])
                    out.append(bi.ins)
                si.on_wait = waits[len(waits) - cap:]
            out.append(inst)
        bb.instructions[:] = out


def _build_nc():
    nc = bass.Bass()
    hsT = nc.dram_tensor("hsT", [D, S], BF16, kind="ExternalInput")
    wqT = nc.dram_tensor("wqT", [D, D], BF16, kind="ExternalInput")
    wkT = nc.dram_tensor("wkT", [D, D], BF16, kind="ExternalInput")
    wvT = nc.dram_tensor("wvT", [D, VW], BF16, kind="ExternalInput")
    bq = nc.dram_tensor("bq", [128, NC_D], F32, kind="ExternalInput")
    bk = nc.dram_tensor("bk", [128, NC_D], F32, kind="ExternalInput")
    bvaug = nc.dram_tensor("bvaug", [128, VW], F32, kind="ExternalInput")
    promptT = nc.dram_tensor("promptT", [D, AT], BF16, kind="ExternalInput")
    mask = nc.dram_tensor("mask", [128, NC_S], F32, kind="ExternalInput")
    gating = nc.dram_tensor("gating", [128, VW], F32, kind="ExternalInput")
    outT = nc.dram_tensor("outT", [D, S], BF16, kind="ExternalOutput")

    with SplitDrainTileContext(nc) as tc:
        _emit(nc, tc, hsT, wqT, wkT, wvT, bq, bk, bvaug, promptT, mask,
              gating, outT)
    _split_sync_waits(nc)
    return nc


def _emit(nc, tc, hsT, wqT, wkT, wvT, bq, bk, bvaug, promptT, mask, gating,
          outT):
    from contextlib import ExitStack

    with ExitStack() as ctx:
        pers = ctx.enter_context(tc.tile_pool(name="pers", bufs=1))

        # ---- SBUF persistent arrays ----
        mask_sb = pers.tile([128, NC_S], F32, tag="mask")
        emask_sb = pers.tile([128, NC_S], F32, tag="emask")
        qT_sb = pers.tile([128, NC_D * S], BF16, tag="qT")
        kT_sb = pers.tile([128, NC_D * S], BF16, tag="kT")
        v_sb = pers.tile([128, NC_S * VW], BF16, tag="v")
        pkT_sb = pers.tile([128, NC_D * AT], BF16, tag="pkT")
        pv_sb = pers.tile([128, VW], BF16, tag="pv")
        hsT_sb = pers.tile([128, NC_D * S], BF16, tag="hsT")
        wqT_sb = pers.tile([128, NC_D * D], BF16, tag="wqT")
        wkT_sb = pers.tile([128, NC_D * D], BF16, tag="wkT")
        wvT_sb = pers.tile([128, NC_D * VW], BF16, tag="wvT")
        pT_sb = pers.tile([128, NC_D * AT], BF16, tag="pT")
        bq_sb = pers.tile([128, NC_D], F32, tag="bq")
        bk_sb = pers.tile([128, NC_D], F32, tag="bk")
        bvaug_sb = pers.tile([128, VW], F32, tag="bvaug")
        gbc_sb = pers.tile([128, VW], F32, tag="gbc")
        pvtmp_sb = pers.tile([64, VW], F32, tag="pvtmp")

        # ---- SBUF working pools ----
        exp_pool = ctx.enter_context(tc.tile_pool(name="expp", bufs=20))
        pexp_pool = ctx.enter_context(tc.tile_pool(name="pexpp", bufs=2))
        vt_pool = ctx.enter_context(tc.tile_pool(name="vtp", bufs=2))
        ce_pool = ctx.enter_context(tc.tile_pool(name="cep", bufs=2))
        nrm_pool = ctx.enter_context(tc.tile_pool(name="nrmp", bufs=4))
        rbc_pool = ctx.enter_context(tc.tile_pool(name="rbcp", bufs=2))
        ot_pool = ctx.enter_context(tc.tile_pool(name="otp", bufs=2))
        dscr_pool = ctx.enter_context(
            tc.tile_pool(name="dscr", bufs=4, space="DRAM"))

        # ---- PSUM pools: 2-slot general (4 banks) + ctx accums (4 banks)
        ps_pool = ctx.enter_context(
            tc.tile_pool(name="psp", bufs=2, space="PSUM"))
        ctx_pool = ctx.enter_context(
            tc.tile_pool(name="ctxp", bufs=2, space="PSUM"))

        # ---- PE warmup first: memset (no deps) then dummy matmuls so the
        # HAM clock gate opens during the input-DMA window ----
        dummy_sb = pers.tile([128, 640], BF16, tag="dummy")
        nc.vector.memset(dummy_sb[:], 0.5)
        wps = ps_pool.tile([128, S], F32, tag="ps", name="warm")
        for i in range(16):
            nc.tensor.matmul(wps[:, 0:512], dummy_sb[:, 0:128],
                             dummy_sb[:, 128:640])

        # ---- input DMAs, critical-path first ----
        def dma_w_slice(dst_sb, src, c0, c1):
            dst = dst_sb[:].rearrange("p (k j) -> p k j", j=D)[
                :, :, c0 * 128 : c1 * 128]
            src_ap = src[:, c0 * 128 : c1 * 128].rearrange(
                "(k p) j -> p k j", p=128)
            nc.sync.dma_start(dst, src_ap)

        dma_w_slice(wkT_sb, wkT, 0, 1)
        dma_w_slice(wqT_sb, wqT, 0, 1)
        nc.sync.dma_start(
            hsT_sb[:].rearrange("p (c s) -> p c s", s=S),
            hsT[:, :].rearrange("(c p) s -> p c s", p=128))
        # host pre-packs bq/bk/mask in SBUF layout (contiguous rows) — as
        # [768,1]-style 4-byte-element DMAs these hog the queues and delayed
        # the hsT transfer (K0 start) by ~4us.
        nc.sync.dma_start(bq_sb[:], bq[:, :])
        nc.sync.dma_start(bk_sb[:], bk[:, :])
        dma_w_slice(wkT_sb, wkT, 1, NC_D)
        dma_w_slice(wqT_sb, wqT, 1, NC_D)
        nc.sync.dma_start(
            wvT_sb[:].rearrange("p (c s) -> p c s", s=VW),
            wvT[:, :].rearrange("(c p) s -> p c s", p=128))
        nc.sync.dma_start(
            pT_sb[:].rearrange("p (c s) -> p c s", s=AT),
            promptT[:, :].rearrange("(c p) s -> p c s", p=128))
        nc.sync.dma_start(mask_sb[:], mask[:, :])
        nc.sync.dma_start(gbc_sb[:], gating[:])
        nc.sync.dma_start(bvaug_sb[:], bvaug[:])

        # warmup ACTs (loads the exp/tanh table set early, off the spine)
        nc.scalar.activation(gbc_sb[:], gbc_sb[:], AF.Tanh)
        ones_slots = gbc_sb[:, :].rearrange(
            "p (h e) -> p h e", h=H)[:, :, DH:DH + 1]
        nc.vector.memset(ones_slots, 1.0)
        nc.scalar.activation(emask_sb[:], mask_sb[:], AF.Exp)

        # ---------------- unit generators ----------------
        def proj_qk(w_sb, b_sb, o_sb, c, half):
            ps = ps_pool.tile([128, S], F32, tag="ps",
                              name=f"qk_{id(w_sb)}_{c}_{half}")
            for kc in range(NC_D):
                nc.tensor.matmul(
                    ps[:, 0:512],
                    w_sb[:, kc * D + c * 128 : kc * D + (c + 1) * 128],
                    hsT_sb[:, kc * S + half * 512 : kc * S + half * 512 + 512],
                    start=(kc == 0), stop=(kc == NC_D - 1))
            nc.vector.tensor_scalar_add(
                o_sb[:, c * S + half * 512 : c * S + half * 512 + 512],
                ps[:, 0:512], b_sb[:, c:c + 1])

        def proj_v(sc, half):
            off, w = (0, 512) if half == 0 else (512, VW - 512)
            ps = ps_pool.tile([128, S], F32, tag="ps", name=f"v_{sc}_{half}")
            for kc in range(NC_D):
                nc.tensor.matmul(
                    ps[:, 0:w],
                    hsT_sb[:, kc * S + sc * 128 : kc * S + (sc + 1) * 128],
                    wvT_sb[:, kc * VW + off : kc * VW + off + w],
                    start=(kc == 0), stop=(kc == NC_D - 1))
            vt = vt_pool.tile([128, 512], F32, tag="vt",
                              name=f"vt_{sc}_{half}")
            nc.vector.tensor_add(vt[:, 0:w], ps[:, 0:w],
                                 bvaug_sb[:, off:off + w])
            nc.vector.tensor_scalar_mul(
                v_sb[:, sc * VW + off : sc * VW + off + w],
                vt[:, 0:w], emask_sb[:, sc:sc + 1])

        def prompt_k(grp):
            cs = range(3 * grp, 3 * grp + 3)
            ps = ps_pool.tile([128, S], F32, tag="ps", name=f"pk_{grp}")
            for i, c in enumerate(cs):
                for kc in range(NC_D):
                    nc.tensor.matmul(
                        ps[:, i * AT : (i + 1) * AT],
                        wkT_sb[:, kc * D + c * 128 : kc * D + (c + 1) * 128],
                        pT_sb[:, kc * AT : (kc + 1) * AT],
                        start=(kc == 0), stop=(kc == NC_D - 1))
            for i, c in enumerate(cs):
                nc.vector.tensor_scalar_add(
                    pkT_sb[:, c * AT : (c + 1) * AT],
                    ps[:, i * AT : (i + 1) * AT], bk_sb[:, c:c + 1])

        def prompt_v(half):
            off, w = (0, 512) if half == 0 else (512, VW - 512)
            ps = ps_pool.tile([128, S], F32, tag="ps", name=f"pv_{half}")
            for kc in range(NC_D):
                nc.tensor.matmul(
                    ps[0:AT, 0:w],
                    pT_sb[:, kc * AT : (kc + 1) * AT],
                    wvT_sb[:, kc * VW + off : kc * VW + off + w],
                    start=(kc == 0), stop=(kc == NC_D - 1))
            nc.vector.tensor_add(pvtmp_sb[:, off:off + w], ps[0:AT, 0:w],
                                 bvaug_sb[0:AT, off:off + w])
            nc.vector.tensor_mul(pv_sb[0:AT, off:off + w],
                                 pvtmp_sb[:, off:off + w],
                                 gbc_sb[0:AT, off:off + w])

        def pv_mirror():
            nc.sync.dma_start(pv_sb[AT:128, :], pv_sb[0:AT, :])

        # scores: one beat = (pair c, tci, sb); tile holds [h0 512 | h1 512]
        def score_beat(c, tci, sb, exps):
            st = ps_pool.tile([128, S], F32, tag="ps",
                              name=f"st_{c}_{tci}_{sb}")
            for h in range(2):
                hp = h * 64
                nc.tensor.matmul(
                    st[:, h * 512 : h * 512 + 512],
                    kT_sb[hp:hp + 64, c * S + tci * 128 : c * S + (tci + 1) * 128],
                    qT_sb[hp:hp + 64, c * S + sb * 512 : c * S + sb * 512 + 512])
            ex = exp_pool.tile([128, S], BF16, tag="exp",
                               name=f"exp_{c}_{tci}_{sb}")
            nc.scalar.activation(ex[:], st[:], AF.Exp, scale=SCALE)
            exps[(tci, sb)] = ex

        # ctx accumulation for (pair c, head-half h) over one tci
        def ctx_unit(c, h, tci, exps, cps):
            lhsT = v_sb[:, tci * VW + (2 * c + h) * 65 :
                        tci * VW + (2 * c + h) * 65 + 65]
            for sb in range(2):
                nc.tensor.matmul(
                    cps[h][:, sb * 512 : (sb + 1) * 512], lhsT,
                    exps[(tci, sb)][:, h * 512 : h * 512 + 512],
                    start=(tci == 0), stop=(tci == NC_S - 1))

        def psp_unit(c):
            psp = ps_pool.tile([128, S], F32, tag="ps", name=f"psp_{c}")
            for sb in range(2):
                for h in range(2):
                    hp = h * 64
                    nc.tensor.matmul(
                        psp[hp:hp + 64, sb * 512 : (sb + 1) * 512],
                        pkT_sb[hp:hp + 64, c * AT : (c + 1) * AT],
                        qT_sb[hp:hp + 64,
                              c * S + sb * 512 : c * S + sb * 512 + 512],
                        tile_position=(hp, hp))
            pexp = pexp_pool.tile([128, S], BF16, tag="pexp",
                                  name=f"pexp_{c}")
            nc.scalar.activation(pexp[:], psp[:], AF.Exp, scale=SCALE)
            return pexp

        # per-pair combine buffer: [65, 4096] f32 =
        #   [h0: ctx 0:1024 | prefix 1024:2048][h1: ctx 2048:3072 | prefix ...]
        # row 64 carries the 4 denominator vectors -> ONE reshape DMA.
        def get_cepe(c, cepes):
            if c not in cepes:
                cepes[c] = ce_pool.tile([65, 4 * S], F32, tag="ce",
                                        name=f"cepe_{c}")
            return cepes[c]

        def pps_unit(c, h, pexp, cepe):
            hp = h * 64
            pps = ps_pool.tile([128, S], F32, tag="ps", name=f"pps_{c}_{h}")
            for sb in range(2):
                nc.tensor.matmul(
                    pps[0:65, sb * 512 : (sb + 1) * 512],
                    pv_sb[hp:hp + 64, (2 * c + h) * 65 : (2 * c + h) * 65 + 65],
                    pexp[hp:hp + 64, sb * 512 : (sb + 1) * 512],
                    tile_position=(hp, 0))
            nc.vector.tensor_copy(
                cepe[0:65, (2 * h + 1) * S : (2 * h + 2) * S], pps[0:65, :])

        def stage1a(c, cps, cepe, engines=None):
            """ctx-psum evacuation — emitted first at a window boundary so
            the ctx pool frees before the next pair's ctx MMs queue."""
            e0, e1 = engines or (nc.vector, nc.vector)
            for h, eng in ((0, e0), (1, e1)):
                dst = cepe[0:65, 2 * h * S : (2 * h + 1) * S]
                if eng is nc.scalar:
                    nc.scalar.copy(dst, cps[h][:])
                else:
                    eng.tensor_copy(dst, cps[h][:])

        def stage1b(c, cepe, q=None):
            """Denominator reciprocal + partition-broadcast via DRAM."""
            q = q or nc.sync
            dresh = nrm_pool.tile([128, 32], F32, tag="dresh",
                                  name=f"dr_{c}")
            nc.sync.dma_start(dresh[:], cepe[64:65, :])
            rrec_f = nrm_pool.tile([128, 32], F32, tag="rrecf",
                                   name=f"rrf_{c}")
            nc.vector.reciprocal(rrec_f[:], dresh[:])
            rrec = nrm_pool.tile([128, 32], BF16, tag="rrec",
                                 name=f"rr_{c}")
            nc.vector.tensor_copy(rrec[:], rrec_f[:])
            r_d = dscr_pool.tile([1, 4 * S], BF16, tag="rd", name=f"rd_{c}")
            q.dma_start(r_d[0:1, :], rrec[:])
            r_bc = rbc_pool.tile([64, 4 * S], BF16, tag="rbc",
                                 name=f"rbc_{c}")
            r_src = bass.AP(r_d[:].tensor, r_d[:].offset,
                            [[0, 64], [1, 4 * S]])
            q.dma_start(r_bc[:], r_src)
            return (cepe, r_bc)

        def stage2(c, state, adds=None, q=None):
            cepe, r_bc = state
            adds = adds or nc.gpsimd
            q = q or nc.sync
            for h in range(2):
                nc.vector.tensor_mul(
                    cepe[0:64, 2 * h * S : (2 * h + 2) * S],
                    cepe[0:64, 2 * h * S : (2 * h + 2) * S],
                    r_bc[:, 2 * h * S : (2 * h + 2) * S])
            ot = ot_pool.tile([64, 2 * S], BF16, tag="ot", name=f"ot_{c}")
            for h in range(2):
                adds.tensor_add(ot[:, h * S : (h + 1) * S],
                                cepe[0:64, 2 * h * S : (2 * h + 1) * S],
                                cepe[0:64, (2 * h + 1) * S : (2 * h + 2) * S])
            dst = outT[2 * c * 64 : (2 * c + 2) * 64, :].rearrange(
                "(h p) s -> p h s", p=64)
            q.dma_start(dst, ot[:].rearrange("p (h s) -> p h s", s=S))

        # ---------------- schedule ----------------
        # startup projections: K0a,Q0a first — score beat 0 only needs the
        # first s-half of each, so it can issue one unit earlier
        for half in range(2):
            proj_qk(wkT_sb, bk_sb, kT_sb, 0, half)
            proj_qk(wqT_sb, bq_sb, qT_sb, 0, half)

        def qk_units(c):
            return [lambda c=c, h=h: proj_qk(wkT_sb, bk_sb, kT_sb, c, h)
                    for h in range(2)] + \
                   [lambda c=c, h=h: proj_qk(wqT_sb, bq_sb, qT_sb, c, h)
                    for h in range(2)]

        proj_sched = {
            0: [lambda: proj_v(0, 0), lambda: proj_v(0, 1),
                lambda: proj_v(1, 0), lambda: proj_v(1, 1),
                lambda: prompt_k(0), lambda: prompt_k(1),
                lambda: proj_v(2, 0), lambda: proj_v(2, 1),
                lambda: prompt_v(0), lambda: prompt_v(1), pv_mirror]
               + qk_units(1),
            1: [lambda sc=sc, h=h: proj_v(sc, h)
                for sc in range(3, 8) for h in range(2)] + qk_units(2),
            2: qk_units(3),
            3: qk_units(4),
            4: qk_units(5),
            5: [],
        }

        exps_by_pair = {}
        pexps = {}
        cps_by_pair = {}
        cepes = {}
        s1_state = {}

        for c in range(PAIRS):
            exps_by_pair[c] = {}
            projq = list(proj_sched[c])
            last = c == PAIRS - 1

            # per-gap plan: gap b runs after score beat b
            plan = {b: [] for b in range(16)}

            def sched(b, fn):
                plan[min(b, 15)].append(fn)

            if c >= 1:
                # prefix chain for pair c-1, spread mid-window: clustering
                # psp/pps at gaps 0-4 starves the 2-slot score rotation at
                # the window boundary (two ~2us ACT stalls per window).
                psp_gap, pps_gaps = (2, (4, 5)) if last else (6, (10, 12))
                sched(psp_gap, lambda pc=c - 1: pexps.__setitem__(
                    pc, psp_unit(pc)))
                sched(pps_gaps[0], lambda pc=c - 1: pps_unit(
                    pc, 0, pexps[pc], get_cepe(pc, cepes)))
                sched(pps_gaps[1], lambda pc=c - 1: pps_unit(
                    pc, 1, pexps[pc], get_cepe(pc, cepes)))
                # ctx(c-1): 2 units per gap; last window front-loads from b0.
                # Window 1 must pace pair-0's ctx BEHIND the V3-V7
                # projection units that pop at gaps 5,6,..,12 — emitting a
                # ctx MM before its v-chunk projection is a stale read.
                cps_by_pair[c - 1] = [
                    ctx_pool.tile([65, S], F32, tag="ctx",
                                  name=f"cps_{c - 1}_{h}")
                    for h in range(2)]
                if c == 1:
                    ctx_gaps = [2, 3, 4, 5, 7, 9, 11, 13]
                elif last:
                    ctx_gaps = [0, 1, 2, 3, 4, 5, 6, 7]
                else:
                    ctx_gaps = [2, 3, 4, 5, 6, 7, 8, 9]
                for tci in range(NC_S):
                    for h in range(2):
                        sched(ctx_gaps[tci],
                              lambda pc=c - 1, h=h, t=tci: ctx_unit(
                                  pc, h, t, exps_by_pair[pc],
                                  cps_by_pair[pc]))
                sa = ctx_gaps[-1] + 1
                sched(sa, lambda pc=c - 1: stage1a(
                    pc, cps_by_pair[pc], get_cepe(pc, cepes)))
                s1b_gap = sa + 1 if last else max(sa + 1, 14)
                sched(s1b_gap, lambda pc=c - 1: s1_state.__setitem__(
                    pc, stage1b(pc, get_cepe(pc, cepes))))
            if c >= 2:
                sched(5, lambda pc=c - 2: stage2(pc, s1_state.pop(pc)))

            if last:
                # pair 5: short-lag in-window ctx + early prefix
                cps_by_pair[5] = [
                    ctx_pool.tile([65, S], F32, tag="ctx",
                                  name=f"cps_5_{h}")
                    for h in range(2)]
                sched(7, lambda: pexps.__setitem__(5, psp_unit(5)))
                for tci in range(NC_S):
                    b = min(tci + 9, 15)
                    for h in range(2):
                        sched(b, lambda h=h, t=tci: ctx_unit(
                            5, h, t, exps_by_pair[5], cps_by_pair[5]))
                sched(12, lambda: pps_unit(
                    5, 0, pexps[5], get_cepe(5, cepes)))
                sched(14, lambda: pps_unit(
                    5, 1, pexps[5], get_cepe(5, cepes)))
                sched(13, lambda: stage2(4, s1_state.pop(4)))

            for b in range(16):
                tci, sb = b // 2, b % 2
                score_beat(c, tci, sb, exps_by_pair[c])
                for fn in plan[b]:
                    fn()
                # proj filler: one unit per free-ish gap; keep the window
                # boundary clear for the score-slot rotation when the proj
                # queue is small enough to afford it
                if projq and not (c >= 1 and b in (6, 10, 12)) \
                        and not (c >= 2 and b < 2):
                    projq.pop(0)()
            while projq:
                projq.pop(0)()

        # ---------------- tail: finish pair 5 ----------------
        # ctx-psum evac split DVE/ACT (ACT idle now); DMA issue spread over
        # the idle PE and GpSimd queues; final adds on DVE so the slow
        # GpSimd drain isn't last.
        stage1a(5, cps_by_pair[5], get_cepe(5, cepes),
                engines=(nc.vector, nc.scalar))
        st5 = stage1b(5, get_cepe(5, cepes), q=nc.scalar)
        stage2(5, st5, adds=nc.vector, q=nc.gpsimd)


def _prep_inputs(hidden_states, prompt_tokens, gating_factor, attention_mask,
                 Wq, bq, Wk, bk, Wv, bv):
    bf = ml_dtypes.bfloat16
    hs = np.asarray(hidden_states, np.float32)
    mask = np.asarray(attention_mask, np.float32).reshape(B, S)
    wqT = np.ascontiguousarray(np.asarray(Wq, np.float32).T).astype(bf)
    wkT = np.ascontiguousarray(np.asarray(Wk, np.float32).T).astype(bf)
    # augmented WvT: [din, 780], col 65h+j = Wv.T[:, 64h+j], col 65h+64 = 0
    wvT_f = np.asarray(Wv, np.float32).T  # [din, dout]
    wvT_aug = np.zeros((D, VW), np.float32)
    idx = np.arange(D)
    aug_cols = (idx // DH) * (DH + 1) + (idx % DH)
    wvT_aug[:, aug_cols] = wvT_f
    wvT_aug = wvT_aug.astype(bf)
    # SBUF layout [128, 6]: col c, partition p = element c*128+p
    bq_c = np.ascontiguousarray(
        np.asarray(bq, np.float32).reshape(NC_D, 128).T)
    bk_c = np.ascontiguousarray(
        np.asarray(bk, np.float32).reshape(NC_D, 128).T)
    bv_aug = np.zeros(VW, np.float32)
    bv_aug[aug_cols] = np.asarray(bv, np.float32)
    bv_aug[DH::DH + 1] = 1.0
    bvaug_bc = np.ascontiguousarray(
        np.broadcast_to(bv_aug, (128, VW)), np.float32)
    pT = np.ascontiguousarray(
        np.asarray(prompt_tokens, np.float32)[0].T).astype(bf)
    gat_row = np.repeat(
        np.asarray(gating_factor, np.float32).reshape(H), DH + 1)
    gat = np.ascontiguousarray(
        np.broadcast_to(gat_row, (128, VW)), np.float32)

    shared = dict(wqT=wqT, wkT=wkT, wvT=wvT_aug, bq=bq_c, bk=bk_c,
                  bvaug=bvaug_bc, promptT=pT, gating=gat)
    in_maps = []
    for b in range(B):
        m = dict(shared)
        m["hsT"] = np.ascontiguousarray(hs[b].T).astype(bf)
        m["mask"] = np.ascontiguousarray(
            mask[b].reshape(NC_S, 128).T.astype(np.float32))
        in_maps.append(m)
    return in_maps


def kernel(**inputs):
    global LAST_RESULTS
    if "nc" not in _CACHE:
        _CACHE["nc"] = _build_nc()
    nc = _CACHE["nc"]
    in_maps = _prep_inputs(**inputs)
    res = None
    for attempt in range(3):
        try:
            res = run_bass_kernel_spmd(nc, in_maps, list(range(B)))
            break
        except ModuleNotFoundError:
            # BASS_TRACE set but this image lacks antenv.axon_hooks
            import os

            os.environ["BASS_NEVER_TRACE"] = "1"
            if attempt == 2:
                raise
        except Exception:
            # transient NRT_EXEC_UNIT_UNRECOVERABLE on a cold device has
            # been observed; a retry on the same session recovers
            if attempt == 2:
                raise
    LAST_RESULTS = res
    out = np.empty((B, S, D), np.float32)
    for b in range(B):
        out[b] = res.results[b]["outT"].T
    return out
